# revision 17
# baseline (speedup 1.0000x reference)
"""Self-contained Trainium2 Bass kernel for nn_MinMaxAttention (lightning-style
block-recurrent linear attention with ALiBi decay + RMS norm + gated output
projection).

Sharding: 8 cores = 2 batches x 4 head-groups (4 heads / 512 channels each).
All GEMMs run in bf16 with fp32 PSUM accumulation; the kv recurrent state is
kept in an fp32 master plus a bf16 matmul copy.

No collectives: a NEFF containing collective_compute runs the PE at ~1.95GHz
instead of 2.35GHz (measured), a 17% clock penalty on everything. Instead each
core returns its unscaled partial out-projection (bf16) plus its per-token
partial sum-of-squares; the host applies 1/sqrt(mean+eps) per token while it
sums the partials (the output is linear in that per-token scale).
"""
import sys
import math

sys.path.insert(0, '/opt/trn_rl_repo')

import numpy as np
import ml_dtypes
import concourse.bass as bass
import concourse.tile as tile
from concourse import bacc, mybir
from concourse.bass_utils import run_bass_kernel_spmd

F32 = mybir.dt.float32
BF16 = mybir.dt.bfloat16
AF = mybir.ActivationFunctionType
BF16_NP = ml_dtypes.bfloat16

NUM_HEADS = 16
HEAD_DIM = 128
BLOCK = 256
EPS = 1e-6
B_BATCH = 2
N_TOK = 4096
D_IN = 2048
D_OUT = 2048
H_CORE = 4           # heads per core
C_CORE = H_CORE * HEAD_DIM   # hidden channels per core (512)
NB = N_TOK // BLOCK  # 16 attention blocks
KC = D_IN // 128     # 16 contraction chunks
N_CORES = 8


def _get_slopes(n):
    def p2(n):
        start = 2 ** (-2 ** (-(math.log2(n) - 3)))
        return [start * start ** i for i in range(n)]
    if math.log2(n).is_integer():
        return p2(n)
    c = 2 ** math.floor(math.log2(n))
    return p2(c) + _get_slopes(2 * c)[0::2][: n - c]


def build_nc(trace_friendly=False):
    nc = bacc.Bacc("TRN2", target_bir_lowering=False, debug=False,
                   num_devices=N_CORES)

    # ---- I/O ----
    xT_d = nc.dram_tensor("xT", [D_IN, N_TOK], BF16, kind="ExternalInput")
    wq_d = nc.dram_tensor("wq", [D_IN, C_CORE], BF16, kind="ExternalInput")
    wk_d = nc.dram_tensor("wk", [D_IN, C_CORE], BF16, kind="ExternalInput")
    wv_d = nc.dram_tensor("wv", [D_IN, C_CORE], BF16, kind="ExternalInput")
    wg_d = nc.dram_tensor("wg", [D_IN, C_CORE], BF16, kind="ExternalInput")
    wout_d = nc.dram_tensor("wout", [C_CORE, D_OUT], BF16,
                            kind="ExternalInput")
    dmask_d = nc.dram_tensor("dmask", [H_CORE, 2, 128, BLOCK], BF16,
                             kind="ExternalInput")
    qdec_d = nc.dram_tensor("qdec", [128, H_CORE, BLOCK], BF16,
                            kind="ExternalInput")
    kdec_d = nc.dram_tensor("kdec", [128, H_CORE, 2], F32,
                            kind="ExternalInput")
    bdec_d = nc.dram_tensor("bdec", [128, H_CORE, 1], F32,
                            kind="ExternalInput")
    iden_d = nc.dram_tensor("iden", [128, 128], BF16, kind="ExternalInput")
    out_d = nc.dram_tensor("out", [N_TOK, D_OUT], BF16,
                           kind="ExternalOutput")
    # per-block channel-major partial sum of o^2: [128 ch, block, 256 tok]
    ssq_d = nc.dram_tensor("ssq", [128, NB, BLOCK], F32,
                           kind="ExternalOutput")

    with tile.TileContext(nc) as tc:
        with (
            tc.tile_pool(name="wpool", bufs=1) as wpool,
            tc.tile_pool(name="cpool", bufs=1) as cpool,
            tc.tile_pool(name="state", bufs=1) as state,
            tc.tile_pool(name="sbA", bufs=2) as sbA,
            tc.tile_pool(name="psP", bufs=1, space="PSUM") as psP,
            tc.tile_pool(name="psA", bufs=1, space="PSUM") as psA,
        ):
            # first attention block of x, before weights hit the queues
            xT_r = xT_d.rearrange("(kc p) n -> p kc n", p=128)
            xT_first = sbA.tile([128, KC, BLOCK], BF16, tag="xT", bufs=3)
            nc.sync.dma_start(out=xT_first[:], in_=xT_r[:, :, 0:BLOCK])

            wq_sb = [wpool.tile([128, C_CORE], BF16, name=f"wq{k}")
                     for k in range(KC)]
            wk_sb = [wpool.tile([128, C_CORE], BF16, name=f"wk{k}")
                     for k in range(KC)]
            wv_sb = [wpool.tile([128, C_CORE], BF16, name=f"wv{k}")
                     for k in range(KC)]
            wg_sb = [wpool.tile([128, C_CORE], BF16, name=f"wg{k}")
                     for k in range(KC)]
            for k in range(KC):
                nc.scalar.dma_start(out=wq_sb[k][:],
                                    in_=wq_d[bass.ts(k, 128), :])
                nc.scalar.dma_start(out=wk_sb[k][:],
                                    in_=wk_d[bass.ts(k, 128), :])
                nc.sync.dma_start(out=wv_sb[k][:],
                                  in_=wv_d[bass.ts(k, 128), :])
            for k in range(KC):
                eng = nc.scalar if k % 2 == 0 else nc.sync
                eng.dma_start(out=wg_sb[k][:],
                              in_=wg_d[bass.ts(k, 128), :])
            wout_sb = [wpool.tile([128, D_OUT], BF16, name=f"wo{h}")
                       for h in range(H_CORE)]
            for h in range(H_CORE):
                nc.scalar.dma_start(out=wout_sb[h][:],
                                    in_=wout_d[bass.ts(h, 128), :])

            dmask_sb = cpool.tile([128, H_CORE, 2, BLOCK], BF16)
            for h in range(H_CORE):
                nc.sync.dma_start(out=dmask_sb[:, h, :, :],
                                   in_=dmask_d[h].rearrange("n p m -> p n m"))
            qdec_sb = cpool.tile([128, H_CORE, BLOCK], BF16)
            nc.scalar.dma_start(out=qdec_sb[:], in_=qdec_d[:])
            kdec_sb = cpool.tile([128, H_CORE, 2], F32)
            nc.scalar.dma_start(out=kdec_sb[:], in_=kdec_d[:])
            bdec_sb = cpool.tile([128, H_CORE, 1], F32)
            nc.scalar.dma_start(out=bdec_sb[:], in_=bdec_d[:])
            iden_sb = cpool.tile([128, 128], BF16)
            nc.scalar.dma_start(out=iden_sb[:], in_=iden_d[:])

            kv32 = state.tile([128, H_CORE, HEAD_DIM], F32)
            kv_bf = state.tile([128, H_CORE, HEAD_DIM], BF16)
            nc.vector.memset(kv32[:], 0.0)
            nc.vector.memset(kv_bf[:], 0.0)

            og_sb = state.tile([128, H_CORE, N_TOK], BF16)

            def out_proj(b):
                """Partial out projection for block b, unscaled, bf16."""
                for t2 in range(2):
                    m = 2 * b + t2
                    og_sl = slice(b * BLOCK + t2 * 128,
                                  b * BLOCK + t2 * 128 + 128)
                    for oc in range(4):
                        op_ps = psP.tile([128, 512], F32, tag="proj", bufs=4)
                        for h in range(H_CORE):
                            nc.tensor.matmul(
                                out=op_ps[:],
                                lhsT=og_sb[:, h, og_sl],
                                rhs=wout_sb[h][:, bass.ts(oc, 512)],
                                start=(h == 0), stop=(h == H_CORE - 1))
                        ot = sbA.tile([128, 512], BF16, tag="out", bufs=6)
                        nc.scalar.activation(out=ot[:], in_=op_ps[:],
                                             func=AF.Copy)
                        nc.sync.dma_start(
                            out=out_d[bass.ts(m, 128), bass.ts(oc, 512)],
                            in_=ot[:])

            for j in range(NB):
                tsl = bass.ts(j, BLOCK)
                if j == 0:
                    xT_blk = xT_first
                else:
                    xT_blk = sbA.tile([128, KC, BLOCK], BF16, tag="xT",
                                      bufs=3)
                    nc.sync.dma_start(out=xT_blk[:], in_=xT_r[:, :, tsl])

                qT_s = sbA.tile([128, H_CORE, BLOCK], BF16, tag="qT", bufs=2)
                qsc_s = sbA.tile([128, H_CORE, BLOCK], BF16, tag="qsc",
                                 bufs=2)
                kT_s = sbA.tile([128, H_CORE, BLOCK], BF16, tag="kT", bufs=2)
                v_s = sbA.tile([128, 2, C_CORE], BF16, tag="v", bufs=2)
                gt_s = sbA.tile([128, H_CORE, BLOCK], BF16, tag="gt", bufs=2)

                for h in range(H_CORE):
                    hsl = bass.ts(h, HEAD_DIM)
                    q_ps = psP.tile([128, BLOCK], F32, tag="proj", bufs=4)
                    for k in range(KC):
                        nc.tensor.matmul(out=q_ps[:], lhsT=wq_sb[k][:, hsl],
                                         rhs=xT_blk[:, k, :],
                                         start=(k == 0), stop=(k == KC - 1))
                    nc.scalar.activation(out=qT_s[:, h, :], in_=q_ps[:],
                                         func=AF.Silu)
                    nc.vector.tensor_mul(qsc_s[:, h, :], qT_s[:, h, :],
                                         qdec_sb[:, h, :])
                    k_ps = psP.tile([128, BLOCK], F32, tag="proj", bufs=4)
                    for k in range(KC):
                        nc.tensor.matmul(out=k_ps[:], lhsT=wk_sb[k][:, hsl],
                                         rhs=xT_blk[:, k, :],
                                         start=(k == 0), stop=(k == KC - 1))
                    nc.scalar.activation(out=kT_s[:, h, :], in_=k_ps[:],
                                         func=AF.Silu)
                for t2 in range(2):
                    v_ps = psP.tile([128, C_CORE], F32, tag="proj", bufs=4)
                    for k in range(KC):
                        nc.tensor.matmul(out=v_ps[:],
                                         lhsT=xT_blk[:, k, bass.ts(t2, 128)],
                                         rhs=wv_sb[k][:],
                                         start=(k == 0), stop=(k == KC - 1))
                    nc.scalar.activation(out=v_s[:, t2, :], in_=v_ps[:],
                                         func=AF.Silu)
                for h in range(H_CORE):
                    hsl = bass.ts(h, HEAD_DIM)
                    g_ps = psP.tile([128, BLOCK], F32, tag="proj", bufs=4)
                    for k in range(KC):
                        nc.tensor.matmul(out=g_ps[:], lhsT=wg_sb[k][:, hsl],
                                         rhs=xT_blk[:, k, :],
                                         start=(k == 0), stop=(k == KC - 1))
                    nc.scalar.activation(out=gt_s[:, h, :], in_=g_ps[:],
                                         func=AF.Sigmoid)

                # -------- attention --------
                sqsum = sbA.tile([128, BLOCK], F32, tag="sqs", bufs=2)
                for h in range(H_CORE):
                    hsl = bass.ts(h, HEAD_DIM)
                    # inter-block term via decayed q against kv state
                    o_ps = psA.tile([128, BLOCK], F32, tag="ops", bufs=2)
                    nc.tensor.matmul(out=o_ps[:], lhsT=kv_bf[:, h, :],
                                     rhs=qsc_s[:, h, :],
                                     start=True, stop=False)
                    # intra-block causal decayed attention
                    for n2 in range(2):
                        qk_ps = psA.tile([128, BLOCK], F32, tag="qk", bufs=2)
                        nc.tensor.matmul(out=qk_ps[:],
                                         lhsT=kT_s[:, h, bass.ts(n2, 128)],
                                         rhs=qT_s[:, h, :],
                                         start=True, stop=True)
                        qkm = sbA.tile([128, BLOCK], BF16, tag="qkm", bufs=2)
                        nc.vector.tensor_mul(qkm[:], qk_ps[:],
                                             dmask_sb[:, h, n2, :])
                        nc.tensor.matmul(out=o_ps[:], lhsT=v_s[:, n2, hsl],
                                         rhs=qkm[:],
                                         start=False, stop=(n2 == 1))
                    # per-token sum of squares, accumulated across heads
                    if h == 0:
                        nc.scalar.square(sqsum[:], o_ps[:])
                    else:
                        sq_t = sbA.tile([128, BLOCK], F32, tag="sq", bufs=2)
                        nc.scalar.square(sq_t[:], o_ps[:])
                        nc.vector.tensor_add(sqsum[:], sqsum[:], sq_t[:])
                    # gated output, bf16, SBUF-resident
                    nc.vector.tensor_mul(og_sb[:, h, tsl], o_ps[:],
                                         gt_s[:, h, :])
                    # kv state update
                    kv_ps = psA.tile([128, HEAD_DIM], F32, tag="ops", bufs=2)
                    for n2 in range(2):
                        kt_ps = psA.tile([128, 128], BF16, tag="qk", bufs=2)
                        nc.tensor.transpose(kt_ps[:],
                                            kT_s[:, h, bass.ts(n2, 128)],
                                            iden_sb[:])
                        ksc = sbA.tile([128, 128], BF16, tag="ksc", bufs=2)
                        nc.vector.tensor_scalar_mul(
                            ksc[:], kt_ps[:], kdec_sb[:, h, n2:n2 + 1])
                        nc.tensor.matmul(out=kv_ps[:], lhsT=ksc[:],
                                         rhs=v_s[:, n2, hsl],
                                         start=(n2 == 0), stop=(n2 == 1))
                    nc.vector.tensor_scalar_mul(kv32[:, h, :], kv32[:, h, :],
                                                bdec_sb[:, h, :])
                    nc.vector.tensor_add(kv32[:, h, :], kv32[:, h, :],
                                         kv_ps[:])
                    nc.vector.tensor_copy(out=kv_bf[:, h, :],
                                          in_=kv32[:, h, :])

                # ship this block's ssq partial (host reduces over channels)
                nc.sync.dma_start(out=ssq_d[:, j, :], in_=sqsum[:])

                # partial out-projection, skewed 2 blocks to decouple
                # the PE from this block's og DVE stream
                if j >= 2:
                    out_proj(j - 2)

            out_proj(NB - 2)
            out_proj(NB - 1)

    nc.compile()
    return nc


_NC_CACHE = {}


def _get_nc():
    if "nc" not in _NC_CACHE:
        _NC_CACHE["nc"] = build_nc()
    return _NC_CACHE["nc"]


def make_in_maps(x, Wqkv, Wg, Wout, norm_w):
    slopes = np.asarray(_get_slopes(NUM_HEADS), dtype=np.float64)
    arr = np.arange(BLOCK, dtype=np.float64) + 1.0
    p_idx = np.arange(128)
    m_idx = np.arange(BLOCK)

    iden = np.eye(128, dtype=BF16_NP)
    wout_scaled = (np.asarray(norm_w)[:, None] * np.asarray(Wout))

    xT_cache = {}
    in_maps = []
    for c in range(N_CORES):
        bi, hg = c // 4, c % 4
        heads = [hg * H_CORE + i for i in range(H_CORE)]
        if bi not in xT_cache:
            xT_cache[bi] = np.ascontiguousarray(
                np.asarray(x[bi]).T.astype(BF16_NP))
        wq = np.concatenate(
            [Wqkv[:, h * 384:h * 384 + 128] for h in heads], axis=1)
        wk = np.concatenate(
            [Wqkv[:, h * 384 + 128:h * 384 + 256] for h in heads], axis=1)
        wv = np.concatenate(
            [Wqkv[:, h * 384 + 256:h * 384 + 384] for h in heads], axis=1)
        wg = Wg[:, hg * C_CORE:(hg + 1) * C_CORE]
        wout = wout_scaled[hg * C_CORE:(hg + 1) * C_CORE, :]

        dmask = np.zeros((H_CORE, 2, 128, BLOCK), dtype=np.float32)
        qdec = np.zeros((128, H_CORE, BLOCK), dtype=np.float32)
        kdec = np.zeros((128, H_CORE, 2), dtype=np.float32)
        bdec = np.zeros((128, H_CORE, 1), dtype=np.float32)
        for i, h in enumerate(heads):
            s = slopes[h]
            for n2 in range(2):
                n_idx = n2 * 128 + p_idx
                diff = m_idx[None, :] - n_idx[:, None]
                dmask[i, n2] = np.where(
                    diff >= 0, np.exp(-s * diff), 0.0).astype(np.float32)
                kdec[:, i, n2] = np.exp(-s * (BLOCK - (n_idx + 1.0)))
            qdec[:, i, :] = np.exp(-s * arr)[None, :]
            bdec[:, i, 0] = math.exp(-s * BLOCK)

        in_maps.append({
            "xT": xT_cache[bi],
            "wq": np.ascontiguousarray(wq).astype(BF16_NP),
            "wk": np.ascontiguousarray(wk).astype(BF16_NP),
            "wv": np.ascontiguousarray(wv).astype(BF16_NP),
            "wg": np.ascontiguousarray(wg).astype(BF16_NP),
            "wout": np.ascontiguousarray(wout).astype(BF16_NP),
            "dmask": dmask.astype(BF16_NP),
            "qdec": qdec.astype(BF16_NP),
            "kdec": kdec,
            "bdec": bdec,
            "iden": iden,
        })
    return in_maps


def kernel(x, Wqkv, Wg, Wout, norm_w, _trace=False, _trace_kwargs=None):
    x = np.asarray(x)
    in_maps = make_in_maps(np.asarray(x), np.asarray(Wqkv), np.asarray(Wg),
                           np.asarray(Wout), np.asarray(norm_w))
    nc = _get_nc()
    res = run_bass_kernel_spmd(nc, in_maps, list(range(N_CORES)),
                               trace=_trace, **(_trace_kwargs or {}))
    out = np.zeros((B_BATCH, N_TOK, D_OUT), dtype=np.float32)
    ssq = np.zeros((B_BATCH, N_TOK), dtype=np.float32)
    for c in range(N_CORES):
        bi = c // 4
        out[bi] += np.asarray(res.results[c]["out"], dtype=np.float32)
        # ssq_d is [128 ch, block, 256 tok]: reduce channels, flatten tokens
        s = np.asarray(res.results[c]["ssq"], dtype=np.float32)
        ssq[bi] += s.sum(axis=0).reshape(N_TOK)
    inv = 1.0 / np.sqrt(ssq / D_OUT + EPS)
    out *= inv[:, :, None]
    kernel._last_results = res
    return out


# revision 19
# speedup vs baseline: 1.1918x; 1.1918x over previous
"""Self-contained Trainium2 Bass kernel for nn_MinMaxAttention (lightning-style
block-recurrent linear attention with ALiBi decay + RMS norm + gated output
projection).

Sharding: 8 cores = 2 batches x 4 head-groups (4 heads / 512 channels each).
All GEMMs run in bf16 with fp32 PSUM accumulation; the kv recurrent state is
kept in an fp32 master plus a bf16 matmul copy.

No collectives: a NEFF containing collective_compute runs the PE at ~1.95GHz
instead of 2.35GHz (measured), a 17% clock penalty on everything. Instead each
core returns its unscaled partial out-projection (bf16) plus its per-token
partial sum-of-squares; the host applies 1/sqrt(mean+eps) per token while it
sums the partials (the output is linear in that per-token scale).
"""
import sys
import math

sys.path.insert(0, '/opt/trn_rl_repo')

import numpy as np
import ml_dtypes
import concourse.bass as bass
import concourse.tile as tile
from concourse import bacc, mybir
from concourse.bass_utils import run_bass_kernel_spmd

F32 = mybir.dt.float32
BF16 = mybir.dt.bfloat16
AF = mybir.ActivationFunctionType
BF16_NP = ml_dtypes.bfloat16

NUM_HEADS = 16
HEAD_DIM = 128
BLOCK = 256
EPS = 1e-6
B_BATCH = 2
N_TOK = 4096
D_IN = 2048
D_OUT = 2048
H_CORE = 4           # heads per core
C_CORE = H_CORE * HEAD_DIM   # hidden channels per core (512)
NB = N_TOK // BLOCK  # 16 attention blocks
KC = D_IN // 128     # 16 contraction chunks
N_CORES = 8


def _get_slopes(n):
    def p2(n):
        start = 2 ** (-2 ** (-(math.log2(n) - 3)))
        return [start * start ** i for i in range(n)]
    if math.log2(n).is_integer():
        return p2(n)
    c = 2 ** math.floor(math.log2(n))
    return p2(c) + _get_slopes(2 * c)[0::2][: n - c]


def build_nc(trace_friendly=False):
    nc = bacc.Bacc("TRN2", target_bir_lowering=False, debug=False,
                   num_devices=N_CORES)

    # ---- I/O ----
    xT_d = nc.dram_tensor("xT", [D_IN, N_TOK], BF16, kind="ExternalInput")
    wq_d = nc.dram_tensor("wq", [D_IN, C_CORE], BF16, kind="ExternalInput")
    wk_d = nc.dram_tensor("wk", [D_IN, C_CORE], BF16, kind="ExternalInput")
    wv_d = nc.dram_tensor("wv", [D_IN, C_CORE], BF16, kind="ExternalInput")
    wg_d = nc.dram_tensor("wg", [D_IN, C_CORE], BF16, kind="ExternalInput")
    wout_d = nc.dram_tensor("wout", [C_CORE, D_OUT], BF16,
                            kind="ExternalInput")
    dmask_d = nc.dram_tensor("dmask", [H_CORE, 2, 128, BLOCK], BF16,
                             kind="ExternalInput")
    qdec_d = nc.dram_tensor("qdec", [128, H_CORE, BLOCK], BF16,
                            kind="ExternalInput")
    kdec_d = nc.dram_tensor("kdec", [128, H_CORE, 2], F32,
                            kind="ExternalInput")
    bdec_d = nc.dram_tensor("bdec", [128, H_CORE, 1], F32,
                            kind="ExternalInput")
    iden_d = nc.dram_tensor("iden", [128, 128], BF16, kind="ExternalInput")
    out_d = nc.dram_tensor("out", [N_TOK, D_OUT], BF16,
                           kind="ExternalOutput")
    # per-block channel-major partial sum of o^2: [128 ch, block, 256 tok]
    ssq_d = nc.dram_tensor("ssq", [128, NB, BLOCK], F32,
                           kind="ExternalOutput")

    with tile.TileContext(nc) as tc:
        with (
            tc.tile_pool(name="wpool", bufs=1) as wpool,
            tc.tile_pool(name="cpool", bufs=1) as cpool,
            tc.tile_pool(name="state", bufs=1) as state,
            tc.tile_pool(name="sbA", bufs=2) as sbA,
            tc.tile_pool(name="psP", bufs=1, space="PSUM") as psP,
            tc.tile_pool(name="psA", bufs=1, space="PSUM") as psA,
        ):
            # first attention block of x, before weights hit the queues
            xT_r = xT_d.rearrange("(kc p) n -> p kc n", p=128)
            xT_first = sbA.tile([128, KC, BLOCK], BF16, tag="xT", bufs=3)
            nc.sync.dma_start(out=xT_first[:], in_=xT_r[:, :, 0:BLOCK])

            wq_sb = [wpool.tile([128, C_CORE], BF16, name=f"wq{k}")
                     for k in range(KC)]
            wk_sb = [wpool.tile([128, C_CORE], BF16, name=f"wk{k}")
                     for k in range(KC)]
            wv_sb = [wpool.tile([128, C_CORE], BF16, name=f"wv{k}")
                     for k in range(KC)]
            wg_sb = [wpool.tile([128, C_CORE], BF16, name=f"wg{k}")
                     for k in range(KC)]
            for k in range(KC):
                nc.scalar.dma_start(out=wq_sb[k][:],
                                    in_=wq_d[bass.ts(k, 128), :])
                nc.scalar.dma_start(out=wk_sb[k][:],
                                    in_=wk_d[bass.ts(k, 128), :])
                nc.sync.dma_start(out=wv_sb[k][:],
                                  in_=wv_d[bass.ts(k, 128), :])
                nc.sync.dma_start(out=wg_sb[k][:],
                                  in_=wg_d[bass.ts(k, 128), :])
            wout_sb = [wpool.tile([128, D_OUT], BF16, name=f"wo{h}")
                       for h in range(H_CORE)]
            for h in range(H_CORE):
                nc.scalar.dma_start(out=wout_sb[h][:],
                                    in_=wout_d[bass.ts(h, 128), :])

            dmask_sb = cpool.tile([128, H_CORE, 2, BLOCK], BF16)
            for h in range(H_CORE):
                nc.sync.dma_start(out=dmask_sb[:, h, :, :],
                                   in_=dmask_d[h].rearrange("n p m -> p n m"))
            qdec_sb = cpool.tile([128, H_CORE, BLOCK], BF16)
            nc.scalar.dma_start(out=qdec_sb[:], in_=qdec_d[:])
            kdec_sb = cpool.tile([128, H_CORE, 2], F32)
            nc.scalar.dma_start(out=kdec_sb[:], in_=kdec_d[:])
            bdec_sb = cpool.tile([128, H_CORE, 1], F32)
            nc.scalar.dma_start(out=bdec_sb[:], in_=bdec_d[:])
            iden_sb = cpool.tile([128, 128], BF16)
            nc.scalar.dma_start(out=iden_sb[:], in_=iden_d[:])

            kv32 = state.tile([128, H_CORE, HEAD_DIM], F32)
            kv_bf = state.tile([128, H_CORE, HEAD_DIM], BF16)
            nc.vector.memset(kv32[:], 0.0)
            nc.vector.memset(kv_bf[:], 0.0)

            og_sb = state.tile([128, H_CORE, N_TOK], BF16)

            def out_proj(b):
                """Partial out projection for block b, unscaled, bf16."""
                for t2 in range(2):
                    m = 2 * b + t2
                    og_sl = slice(b * BLOCK + t2 * 128,
                                  b * BLOCK + t2 * 128 + 128)
                    for oc in range(4):
                        op_ps = psP.tile([128, 512], F32, tag="proj", bufs=4)
                        for h in range(H_CORE):
                            nc.tensor.matmul(
                                out=op_ps[:],
                                lhsT=og_sb[:, h, og_sl],
                                rhs=wout_sb[h][:, bass.ts(oc, 512)],
                                start=(h == 0), stop=(h == H_CORE - 1))
                        ot = sbA.tile([128, 512], BF16, tag="out", bufs=6)
                        nc.scalar.activation(out=ot[:], in_=op_ps[:],
                                             func=AF.Copy)
                        nc.scalar.dma_start(
                            out=out_d[bass.ts(m, 128), bass.ts(oc, 512)],
                            in_=ot[:])

            for j in range(NB):
                tsl = bass.ts(j, BLOCK)
                if j == 0:
                    xT_blk = xT_first
                else:
                    xT_blk = sbA.tile([128, KC, BLOCK], BF16, tag="xT",
                                      bufs=3)
                    nc.sync.dma_start(out=xT_blk[:], in_=xT_r[:, :, tsl])

                qT_s = sbA.tile([128, H_CORE, BLOCK], BF16, tag="qT", bufs=2)
                qsc_s = sbA.tile([128, H_CORE, BLOCK], BF16, tag="qsc",
                                 bufs=2)
                kT_s = sbA.tile([128, H_CORE, BLOCK], BF16, tag="kT", bufs=2)
                v_s = sbA.tile([128, 2, C_CORE], BF16, tag="v", bufs=2)
                gt_s = sbA.tile([128, H_CORE, BLOCK], BF16, tag="gt", bufs=2)

                for h in range(H_CORE):
                    hsl = bass.ts(h, HEAD_DIM)
                    q_ps = psP.tile([128, BLOCK], F32, tag="proj", bufs=4)
                    for k in range(KC):
                        nc.tensor.matmul(out=q_ps[:], lhsT=wq_sb[k][:, hsl],
                                         rhs=xT_blk[:, k, :],
                                         start=(k == 0), stop=(k == KC - 1))
                    nc.scalar.activation(out=qT_s[:, h, :], in_=q_ps[:],
                                         func=AF.Silu)
                    nc.vector.tensor_mul(qsc_s[:, h, :], qT_s[:, h, :],
                                         qdec_sb[:, h, :])
                    k_ps = psP.tile([128, BLOCK], F32, tag="proj", bufs=4)
                    for k in range(KC):
                        nc.tensor.matmul(out=k_ps[:], lhsT=wk_sb[k][:, hsl],
                                         rhs=xT_blk[:, k, :],
                                         start=(k == 0), stop=(k == KC - 1))
                    nc.scalar.activation(out=kT_s[:, h, :], in_=k_ps[:],
                                         func=AF.Silu)
                for t2 in range(2):
                    v_ps = psP.tile([128, C_CORE], F32, tag="proj", bufs=4)
                    for k in range(KC):
                        nc.tensor.matmul(out=v_ps[:],
                                         lhsT=xT_blk[:, k, bass.ts(t2, 128)],
                                         rhs=wv_sb[k][:],
                                         start=(k == 0), stop=(k == KC - 1))
                    nc.scalar.activation(out=v_s[:, t2, :], in_=v_ps[:],
                                         func=AF.Silu)
                for h in range(H_CORE):
                    hsl = bass.ts(h, HEAD_DIM)
                    g_ps = psP.tile([128, BLOCK], F32, tag="proj", bufs=4)
                    for k in range(KC):
                        nc.tensor.matmul(out=g_ps[:], lhsT=wg_sb[k][:, hsl],
                                         rhs=xT_blk[:, k, :],
                                         start=(k == 0), stop=(k == KC - 1))
                    nc.scalar.activation(out=gt_s[:, h, :], in_=g_ps[:],
                                         func=AF.Sigmoid)

                # -------- attention --------
                sqsum = sbA.tile([128, BLOCK], F32, tag="sqs", bufs=2)
                for h in range(H_CORE):
                    hsl = bass.ts(h, HEAD_DIM)
                    # inter-block term via decayed q against kv state
                    o_ps = psA.tile([128, BLOCK], F32, tag="ops", bufs=2)
                    nc.tensor.matmul(out=o_ps[:], lhsT=kv_bf[:, h, :],
                                     rhs=qsc_s[:, h, :],
                                     start=True, stop=False)
                    # intra-block causal decayed attention
                    for n2 in range(2):
                        qk_ps = psA.tile([128, BLOCK], F32, tag="qk", bufs=2)
                        nc.tensor.matmul(out=qk_ps[:],
                                         lhsT=kT_s[:, h, bass.ts(n2, 128)],
                                         rhs=qT_s[:, h, :],
                                         start=True, stop=True)
                        qkm = sbA.tile([128, BLOCK], BF16, tag="qkm", bufs=2)
                        nc.vector.tensor_mul(qkm[:], qk_ps[:],
                                             dmask_sb[:, h, n2, :])
                        nc.tensor.matmul(out=o_ps[:], lhsT=v_s[:, n2, hsl],
                                         rhs=qkm[:],
                                         start=False, stop=(n2 == 1))
                    # per-token sum of squares, accumulated across heads
                    if h == 0:
                        nc.scalar.square(sqsum[:], o_ps[:])
                    else:
                        sq_t = sbA.tile([128, BLOCK], F32, tag="sq", bufs=2)
                        nc.scalar.square(sq_t[:], o_ps[:])
                        nc.vector.tensor_add(sqsum[:], sqsum[:], sq_t[:])
                    # gated output, bf16, SBUF-resident
                    nc.vector.tensor_mul(og_sb[:, h, tsl], o_ps[:],
                                         gt_s[:, h, :])
                    # kv state update
                    kv_ps = psA.tile([128, HEAD_DIM], F32, tag="ops", bufs=2)
                    for n2 in range(2):
                        kt_ps = psA.tile([128, 128], BF16, tag="qk", bufs=2)
                        nc.tensor.transpose(kt_ps[:],
                                            kT_s[:, h, bass.ts(n2, 128)],
                                            iden_sb[:])
                        ksc = sbA.tile([128, 128], BF16, tag="ksc", bufs=2)
                        nc.vector.tensor_scalar_mul(
                            ksc[:], kt_ps[:], kdec_sb[:, h, n2:n2 + 1])
                        nc.tensor.matmul(out=kv_ps[:], lhsT=ksc[:],
                                         rhs=v_s[:, n2, hsl],
                                         start=(n2 == 0), stop=(n2 == 1))
                    nc.vector.tensor_scalar_mul(kv32[:, h, :], kv32[:, h, :],
                                                bdec_sb[:, h, :])
                    nc.vector.tensor_add(kv32[:, h, :], kv32[:, h, :],
                                         kv_ps[:])
                    nc.vector.tensor_copy(out=kv_bf[:, h, :],
                                          in_=kv32[:, h, :])

                # ship this block's ssq partial (host reduces over channels)
                nc.sync.dma_start(out=ssq_d[:, j, :], in_=sqsum[:])

                # partial out-projection, skewed 2 blocks to decouple
                # the PE from this block's og DVE stream
                if j >= 2:
                    out_proj(j - 2)

            out_proj(NB - 2)
            out_proj(NB - 1)

    nc.compile()
    return nc


_NC_CACHE = {}


def _get_nc():
    if "nc" not in _NC_CACHE:
        _NC_CACHE["nc"] = build_nc()
    return _NC_CACHE["nc"]


def make_in_maps(x, Wqkv, Wg, Wout, norm_w):
    slopes = np.asarray(_get_slopes(NUM_HEADS), dtype=np.float64)
    arr = np.arange(BLOCK, dtype=np.float64) + 1.0
    p_idx = np.arange(128)
    m_idx = np.arange(BLOCK)

    iden = np.eye(128, dtype=BF16_NP)
    wout_scaled = (np.asarray(norm_w)[:, None] * np.asarray(Wout))

    xT_cache = {}
    in_maps = []
    for c in range(N_CORES):
        bi, hg = c // 4, c % 4
        heads = [hg * H_CORE + i for i in range(H_CORE)]
        if bi not in xT_cache:
            xT_cache[bi] = np.ascontiguousarray(
                np.asarray(x[bi]).T.astype(BF16_NP))
        wq = np.concatenate(
            [Wqkv[:, h * 384:h * 384 + 128] for h in heads], axis=1)
        wk = np.concatenate(
            [Wqkv[:, h * 384 + 128:h * 384 + 256] for h in heads], axis=1)
        wv = np.concatenate(
            [Wqkv[:, h * 384 + 256:h * 384 + 384] for h in heads], axis=1)
        wg = Wg[:, hg * C_CORE:(hg + 1) * C_CORE]
        wout = wout_scaled[hg * C_CORE:(hg + 1) * C_CORE, :]

        dmask = np.zeros((H_CORE, 2, 128, BLOCK), dtype=np.float32)
        qdec = np.zeros((128, H_CORE, BLOCK), dtype=np.float32)
        kdec = np.zeros((128, H_CORE, 2), dtype=np.float32)
        bdec = np.zeros((128, H_CORE, 1), dtype=np.float32)
        for i, h in enumerate(heads):
            s = slopes[h]
            for n2 in range(2):
                n_idx = n2 * 128 + p_idx
                diff = m_idx[None, :] - n_idx[:, None]
                dmask[i, n2] = np.where(
                    diff >= 0, np.exp(-s * diff), 0.0).astype(np.float32)
                kdec[:, i, n2] = np.exp(-s * (BLOCK - (n_idx + 1.0)))
            qdec[:, i, :] = np.exp(-s * arr)[None, :]
            bdec[:, i, 0] = math.exp(-s * BLOCK)

        in_maps.append({
            "xT": xT_cache[bi],
            "wq": np.ascontiguousarray(wq).astype(BF16_NP),
            "wk": np.ascontiguousarray(wk).astype(BF16_NP),
            "wv": np.ascontiguousarray(wv).astype(BF16_NP),
            "wg": np.ascontiguousarray(wg).astype(BF16_NP),
            "wout": np.ascontiguousarray(wout).astype(BF16_NP),
            "dmask": dmask.astype(BF16_NP),
            "qdec": qdec.astype(BF16_NP),
            "kdec": kdec,
            "bdec": bdec,
            "iden": iden,
        })
    return in_maps


def kernel(x, Wqkv, Wg, Wout, norm_w, _trace=False, _trace_kwargs=None):
    x = np.asarray(x)
    in_maps = make_in_maps(np.asarray(x), np.asarray(Wqkv), np.asarray(Wg),
                           np.asarray(Wout), np.asarray(norm_w))
    nc = _get_nc()
    res = run_bass_kernel_spmd(nc, in_maps, list(range(N_CORES)),
                               trace=_trace, **(_trace_kwargs or {}))
    out = np.zeros((B_BATCH, N_TOK, D_OUT), dtype=np.float32)
    ssq = np.zeros((B_BATCH, N_TOK), dtype=np.float32)
    for c in range(N_CORES):
        bi = c // 4
        out[bi] += np.asarray(res.results[c]["out"], dtype=np.float32)
        # ssq_d is [128 ch, block, 256 tok]: reduce channels, flatten tokens
        s = np.asarray(res.results[c]["ssq"], dtype=np.float32)
        ssq[bi] += s.sum(axis=0).reshape(N_TOK)
    inv = 1.0 / np.sqrt(ssq / D_OUT + EPS)
    out *= inv[:, :, None]
    kernel._last_results = res
    return out
